# revision 1
# baseline (speedup 1.0000x reference)
"""Trainium2 Bass kernel for an AttentionBlock (GroupNorm + MHA + proj + residual).

Shapes (hardcoded): x (16, 512, 32, 32) f32, 8 heads (ch=64), GN groups=32,
w_qkv (1536, 512), w_proj (512, 512).

Strategy: data-parallel over batch across 8 NeuronCores (2 batches/core, no
collectives). All large matmuls run in float32r (full PE rate for free dim
>= 256). Scores are computed transposed (s on partitions, t free) so the
softmax denominator falls out of the attn@v matmul as a 65th output row
(ones column appended to v^T); no cross-partition reductions anywhere.
GroupNorm statistics use DVE reductions + a tiny block-diagonal matmul, and
rsqrt is computed with a DVE-only Newton iteration (no ACT table swaps —
ScalarE does nothing but exp, which is the bottleneck engine).

Software pipelining across batches (the ACT/exp stream must not starve):
batch b+1's x DMA (Pool queue) and GroupNorm statistics (DVE-only, cannot
stall the PE) are issued before heads(b) so they execute under it; the rest
of b+1's front-end (GN finish + qkv + v^T) is emitted after heads(b) but
BEFORE proj(b), so the PE covers proj's wait on the last softmax-normalize
with independent work and enters heads(b+1) without a dry ACT queue.
"""
import numpy as np
import ml_dtypes
from contextlib import ExitStack

import concourse.bass as bass
import concourse.mybir as mybir
import concourse.tile as tile
from concourse import bacc
from concourse.bass_utils import run_bass_kernel_spmd

F32 = mybir.dt.float32
F32R = mybir.dt.float32r
BF16 = mybir.dt.bfloat16
AF = mybir.ActivationFunctionType
OP = mybir.AluOpType

B, C, H, W = 16, 512, 32, 32
N = H * W            # 1024
NHEADS = 8
CH = C // NHEADS     # 64
NGROUPS = 32
GSIZE = C // NGROUPS  # 16 channels per group
EPS = 1e-5
NCORES = 8
BPC = B // NCORES    # batches per core = 2
NT = C // 128        # channel tiles per batch = 4
VW = NHEADS * (CH + 1)  # v_ext free width = 520

_cached = {}


def _build(dbg=False, reps=1, num_devices=NCORES):
    nc = bacc.Bacc("TRN2", target_bir_lowering=False, debug=False,
                   num_devices=num_devices)

    xd = nc.dram_tensor("x", [BPC, C, N], F32, kind="ExternalInput").ap()
    wqk_d = nc.dram_tensor("wqk_t", [C, 2 * C], BF16, kind="ExternalInput").ap()
    wv_d = nc.dram_tensor("wv_ext", [C, VW], BF16, kind="ExternalInput").ap()
    wp_d = nc.dram_tensor("wp_t", [C, C], BF16, kind="ExternalInput").ap()
    bqk_d = nc.dram_tensor("bqk", [128, 8], F32, kind="ExternalInput").ap()
    bv_d = nc.dram_tensor("bv_bc", [128, VW], F32, kind="ExternalInput").ap()
    bp_d = nc.dram_tensor("bp", [128, NT], F32, kind="ExternalInput").ap()
    gam_d = nc.dram_tensor("gamma_t", [128, NT], F32, kind="ExternalInput").ap()
    bet_d = nc.dram_tensor("beta_t", [128, NT], F32, kind="ExternalInput").ap()
    bd_d = nc.dram_tensor("blockdiag16", [128, 8], F32, kind="ExternalInput").ap()
    bc_d = nc.dram_tensor("bcast16", [8, 128], F32, kind="ExternalInput").ap()
    outd = nc.dram_tensor("out", [BPC, C, N], F32, kind="ExternalOutput").ap()

    with tile.TileContext(nc) as tc, ExitStack() as ctx:
        wpool = ctx.enter_context(tc.tile_pool(name="weights", bufs=1))
        xpool = ctx.enter_context(tc.tile_pool(name="x", bufs=2))
        xnpool = ctx.enter_context(tc.tile_pool(name="xn", bufs=2))
        qkpool = ctx.enter_context(tc.tile_pool(name="qk", bufs=2))
        vpool = ctx.enter_context(tc.tile_pool(name="v", bufs=2))
        hpool = ctx.enter_context(tc.tile_pool(name="h", bufs=2))
        ppool = ctx.enter_context(tc.tile_pool(name="p", bufs=5))
        opool = ctx.enter_context(tc.tile_pool(name="o", bufs=4))
        small = ctx.enter_context(tc.tile_pool(name="small", bufs=2))
        scr = ctx.enter_context(tc.tile_pool(name="scr", bufs=1))
        ps_sc = ctx.enter_context(tc.tile_pool(name="ps_sc", bufs=2, space="PSUM"))
        ps_h = ctx.enter_context(tc.tile_pool(name="ps_h", bufs=2, space="PSUM"))

        wqk_r, wv_r, wp_r = [], [], []
        for k in range(NT):
            wr = wpool.tile([128, 2 * C], BF16, tag=f"wqk{k}")
            nc.sync.dma_start(wr[:], wqk_d[128 * k:128 * (k + 1), :])
            wqk_r.append(wr)
        for k in range(NT):
            wr = wpool.tile([128, VW], BF16, tag=f"wv{k}")
            nc.sync.dma_start(wr[:], wv_d[128 * k:128 * (k + 1), :])
            wv_r.append(wr)
        for k in range(NT):
            wr = wpool.tile([128, C], BF16, tag=f"wp{k}")
            nc.sync.dma_start(wr[:], wp_d[128 * k:128 * (k + 1), :])
            wp_r.append(wr)

        bqk = wpool.tile([128, 8], F32, tag="bqk")
        nc.sync.dma_start(bqk[:], bqk_d[:])
        bv = wpool.tile([128, VW], F32, tag="bv")
        nc.sync.dma_start(bv[:], bv_d[:])
        bp = wpool.tile([128, NT], F32, tag="bp")
        nc.sync.dma_start(bp[:], bp_d[:])
        gam = wpool.tile([128, NT], F32, tag="gam")
        nc.sync.dma_start(gam[:], gam_d[:])
        bet = wpool.tile([128, NT], F32, tag="bet")
        nc.sync.dma_start(bet[:], bet_d[:])
        bd16 = wpool.tile([128, 8], F32, tag="bd16")
        nc.sync.dma_start(bd16[:], bd_d[:])
        bc16 = wpool.tile([8, 128], F32, tag="bc16")
        nc.sync.dma_start(bc16[:], bc_d[:])

        class St:
            pass

        def emit_load(st):
            # Pool's DGE queue: issued before the previous batch's heads, so
            # the transfer lands while ACT is busy with exp
            st.x_sb = xpool.tile([128, NT * N], F32, tag="x", name="x_sb")
            for j in range(NT):
                nc.gpsimd.dma_start(st.x_sb[:, N * j:N * (j + 1)],
                                    xd[st.b, 128 * j:128 * (j + 1), :])

        def emit_stats(st):
            # DVE-only: runs under the previous batch's heads
            st.stat = small.tile([128, 8], F32, tag="stat", name="stat")
            sq = scr.tile([128, N], F32, tag="sq")
            for j in range(NT):
                nc.vector.reduce_sum(st.stat[:, j:j + 1],
                                     st.x_sb[:, N * j:N * (j + 1)],
                                     axis=mybir.AxisListType.X)
                nc.vector.scalar_tensor_tensor(
                    sq[:], st.x_sb[:, N * j:N * (j + 1)], 1.0,
                    st.x_sb[:, N * j:N * (j + 1)],
                    op0=OP.bypass, op1=OP.mult,
                    accum_out=st.stat[:, 4 + j:5 + j])

        def emit_gn_rest(st):
            ps_st = ps_sc.tile([8, 8], F32, tag="sc")
            nc.tensor.matmul(ps_st[:], bd16[:], st.stat[:], start=True, stop=True)
            inv = 1.0 / (GSIZE * N)
            mean8 = small.tile([8, 8], F32, tag="mean8")
            nc.vector.tensor_scalar_mul(mean8[:, 0:4], ps_st[:, 0:4], inv)
            ex2 = small.tile([8, 4], F32, tag="ex2")
            nc.vector.tensor_scalar_mul(ex2[:], ps_st[:, 4:8], inv)
            m2 = small.tile([8, 4], F32, tag="m2")
            nc.vector.tensor_mul(m2[:], mean8[:, 0:4], mean8[:, 0:4])
            veps = small.tile([8, 4], F32, tag="veps")
            nc.vector.scalar_tensor_tensor(veps[:], ex2[:], EPS, m2[:],
                                           op0=OP.add, op1=OP.subtract)
            r_cur = small.tile([8, 4], F32, tag="r0")
            nc.vector.tensor_scalar(r_cur[:], veps[:], -0.5, 1.5,
                                    op0=OP.mult, op1=OP.add)
            for it in range(3):
                t1 = small.tile([8, 4], F32, tag=f"nt1_{it}")
                nc.vector.tensor_mul(t1[:], r_cur[:], r_cur[:])
                t2 = small.tile([8, 4], F32, tag=f"nt2_{it}")
                nc.vector.scalar_tensor_tensor(t2[:], t1[:], -0.5, veps[:],
                                               op0=OP.mult, op1=OP.mult)
                t3 = small.tile([8, 4], F32, tag=f"nt3_{it}")
                nc.vector.tensor_scalar_add(t3[:], t2[:], 1.5)
                r_nxt = small.tile([8, 4], F32, tag=f"nr_{it}")
                nc.vector.tensor_mul(r_nxt[:], r_cur[:], t3[:])
                r_cur = r_nxt
            nc.vector.tensor_copy(mean8[:, 4:8], r_cur[:])
            ps_bc = ps_sc.tile([128, 8], F32, tag="sc")
            nc.tensor.matmul(ps_bc[:], bc16[:], mean8[:], start=True, stop=True)
            A_ch = small.tile([128, NT], F32, tag="A_ch")
            nc.vector.tensor_mul(A_ch[:], gam[:], ps_bc[:, 4:8])
            tB = small.tile([128, NT], F32, tag="tB")
            nc.vector.tensor_mul(tB[:], ps_bc[:, 0:4], A_ch[:])
            B_ch = small.tile([128, NT], F32, tag="B_ch")
            nc.vector.scalar_tensor_tensor(B_ch[:], tB[:], -1.0, bet[:],
                                           op0=OP.mult, op1=OP.add)
            st.xn = xnpool.tile([128, NT * N], BF16, tag="xn", name="xn")
            for j in range(NT):
                nc.vector.tensor_scalar(st.xn[:, N * j:N * (j + 1)],
                                        st.x_sb[:, N * j:N * (j + 1)],
                                        A_ch[:, j:j + 1], B_ch[:, j:j + 1],
                                        op0=OP.mult, op1=OP.add)

        def emit_qkv(st):
            # qk layout: cols 0..4095 = q (4 ch-tiles), 4096..8191 = k
            st.qk = qkpool.tile([128, 8 * N], BF16, tag="qk", name="qk")
            for o in range(8):
                for nh in range(2):
                    pq = ps_h.tile([128, 512], F32, tag="hacc",
                                   name=f"pq{o}_{nh}")
                    for k in range(NT):
                        nc.tensor.matmul(
                            pq[:],
                            wqk_r[k][:, 128 * o:128 * (o + 1)],
                            st.xn[:, N * k + 512 * nh:N * k + 512 * (nh + 1)],
                            start=(k == 0), stop=(k == NT - 1))
                    nc.vector.tensor_scalar_add(
                        st.qk[:, N * o + 512 * nh:N * o + 512 * (nh + 1)],
                        pq[:], bqk[:, o:o + 1])

        def emit_v(st):
            st.vv = vpool.tile([128, 8 * VW], BF16, tag="vv", name="vv")
            for ntile in range(8):
                for chh in range(2):
                    pv = ps_h.tile([128, 260], F32, tag="hacc",
                                   name=f"pv{ntile}_{chh}")
                    for k in range(NT):
                        nc.tensor.matmul(
                            pv[:],
                            st.xn[:, N * k + 128 * ntile:N * k + 128 * (ntile + 1)],
                            wv_r[k][:, 260 * chh:260 * (chh + 1)],
                            start=(k == 0), stop=(k == NT - 1))
                    nc.vector.tensor_add(
                        st.vv[:, VW * ntile + 260 * chh:
                              VW * ntile + 260 * (chh + 1)],
                        pv[:], bv[:, 260 * chh:260 * (chh + 1)])

        def emit_heads(st):
            qk, vv = st.qk, st.vv
            st.hall = hpool.tile([128, NT * N], BF16, tag="hall", name="hall")
            hall = st.hall

            def make_attnv(phs_, pr_):
                def attnv(side, j, p_tile):
                    head = 2 * pr_ + side
                    for th in range(2):
                        nc.tensor.matmul(
                            phs_[side][:, 512 * th:512 * (th + 1)],
                            vv[:, VW * j + (CH + 1) * head:
                               VW * j + (CH + 1) * head + CH + 1],
                            p_tile[:, 512 * th:512 * (th + 1)],
                            start=(j == 0), stop=(j == 7))
                return attnv

            def emit_tail(tail_):
                attnv_, phs_, p_prev_, pr_ = tail_
                for side in range(2):
                    attnv_(side, 7, p_prev_[side])
                for side in range(2):
                    hc = scr.tile([65, N], F32, tag="hc", bufs=2)
                    nc.vector.tensor_copy(hc[:], phs_[side][:])
                    rec = small.tile([1, N], F32, tag="rec")
                    nc.vector.reciprocal(rec[:], hc[64:65, :])
                    rb = scr.tile([64, N], F32, tag="rb", bufs=2)
                    nc.gpsimd.partition_broadcast(rb[:], rec[:])
                    nc.vector.tensor_mul(
                        hall[64 * side:64 * side + 64, N * pr_:N * (pr_ + 1)],
                        hc[0:64, :], rb[:])

            tail = None
            for pr in range(4):
                q_base = N * pr
                k_base = 4 * N + N * pr
                phs = [ps_h.tile([65, N], F32, tag="hacc", name=f"phA{pr}"),
                       ps_h.tile([65, N], F32, tag="hacc", name=f"phB{pr}")]
                attnv = make_attnv(phs, pr)

                p_prev = [None, None]
                for j in range(8):
                    for side in range(2):
                        pb = 64 * side
                        sc = ps_sc.tile([128, N], F32, tag="sc")
                        for th in range(2):
                            nc.tensor.matmul(
                                sc[:, 512 * th:512 * (th + 1)],
                                qk[pb:pb + 64, k_base + 128 * j:k_base + 128 * (j + 1)],
                                qk[pb:pb + 64, q_base + 512 * th:q_base + 512 * (th + 1)],
                                start=True, stop=True,
                                tile_position=(pb, 0))
                        p_t = ppool.tile([128, N], BF16, tag="p")
                        nc.scalar.activation(p_t[:], sc[:], AF.Exp)
                        if p_prev[side] is not None:
                            attnv(side, j - 1, p_prev[side])
                        p_prev[side] = p_t
                        if j == 0 and side == 1 and tail is not None:
                            emit_tail(tail)
                            tail = None
                tail = (attnv, phs, p_prev, pr)
            emit_tail(tail)

        def emit_proj(st):
            for o in range(NT):
                for nh in range(2):
                    pp = ps_h.tile([128, 512], F32, tag="hacc")
                    for k in range(NT):
                        nc.tensor.matmul(
                            pp[:],
                            wp_r[k][:, 128 * o:128 * (o + 1)],
                            st.hall[:, N * k + 512 * nh:N * k + 512 * (nh + 1)],
                            start=(k == 0), stop=(k == NT - 1))
                    ot = opool.tile([128, 512], F32, tag="ot")
                    nc.vector.scalar_tensor_tensor(
                        ot[:], pp[:], bp[:, o:o + 1],
                        st.x_sb[:, N * o + 512 * nh:N * o + 512 * (nh + 1)],
                        op0=OP.add, op1=OP.add)
                    nc.sync.dma_start(
                        outd[st.b, 128 * o:128 * (o + 1), 512 * nh:512 * (nh + 1)],
                        ot[:])

        bs = [b for _ in range(reps) for b in range(BPC)]
        st = St()
        st.b = bs[0]
        emit_load(st)
        emit_stats(st)
        emit_gn_rest(st)
        emit_qkv(st)
        emit_v(st)
        for idx in range(len(bs)):
            if idx + 1 < len(bs):
                nxt = St()
                nxt.b = bs[idx + 1]
                emit_load(nxt)   # Pool DMA: transfers during heads(st)
                emit_stats(nxt)  # DVE-only: runs under heads(st)
            else:
                nxt = None
            emit_heads(st)
            if nxt is not None:
                # front-end of b+1 before proj(b): the PE covers proj's wait
                # on the last normalize, and heads(b+1) starts with qk ready
                emit_gn_rest(nxt)
                emit_qkv(nxt)
                emit_v(nxt)
            emit_proj(st)
            if nxt is not None:
                st = nxt

    nc.compile()
    return nc


def _prep_shared(w_qkv, b_qkv, w_proj, b_proj, gamma, beta):
    qs = 1.0 / np.sqrt(np.sqrt(float(CH)))  # ch**-0.25
    s2 = qs * qs
    r = np.arange(3 * C).reshape(NHEADS, 3, CH)
    idx_q, idx_k, idx_v = r[:, 0].ravel(), r[:, 1].ravel(), r[:, 2].ravel()
    wqk_t = np.ascontiguousarray(
        np.concatenate([w_qkv[idx_q], w_qkv[idx_k]], axis=0).T).astype(np.float32)
    wqk_t[:, :C] *= s2
    bqk_full = np.concatenate([b_qkv[idx_q], b_qkv[idx_k]])
    bqk_full[:C] *= s2
    bqk = np.ascontiguousarray(bqk_full.reshape(8, 128).T).astype(np.float32)

    wv = w_qkv[idx_v]
    bv_src = b_qkv[idx_v]
    wv_ext = np.zeros((C, VW), np.float32)
    bv_ext = np.zeros((VW,), np.float32)
    for h in range(NHEADS):
        wv_ext[:, (CH + 1) * h:(CH + 1) * h + CH] = wv[CH * h:CH * (h + 1), :].T
        bv_ext[(CH + 1) * h:(CH + 1) * h + CH] = bv_src[CH * h:CH * (h + 1)]
        bv_ext[(CH + 1) * h + CH] = 1.0
    bv_bc = np.ascontiguousarray(np.broadcast_to(bv_ext, (128, VW))).astype(np.float32)

    wp_t = np.ascontiguousarray(w_proj.T).astype(np.float32)
    bp = np.ascontiguousarray(b_proj.reshape(NT, 128).T).astype(np.float32)
    gamma_t = np.ascontiguousarray(gamma.reshape(NT, 128).T).astype(np.float32)
    beta_t = np.ascontiguousarray(beta.reshape(NT, 128).T).astype(np.float32)
    blockdiag16 = np.kron(np.eye(8, dtype=np.float32), np.ones((GSIZE, 1), np.float32))
    bcast16 = np.ascontiguousarray(blockdiag16.T)
    bf = ml_dtypes.bfloat16
    wqk_t, wv_ext, wp_t = (a.astype(bf) for a in (wqk_t, wv_ext, wp_t))
    return dict(wqk_t=wqk_t, bqk=bqk, wv_ext=wv_ext, bv_bc=bv_bc, wp_t=wp_t,
                bp=bp, gamma_t=gamma_t, beta_t=beta_t,
                blockdiag16=blockdiag16, bcast16=bcast16)


def kernel(x, gamma, beta, w_qkv, b_qkv, w_proj, b_proj):
    x = np.asarray(x, dtype=np.float32)
    shared = _prep_shared(np.asarray(w_qkv, np.float32), np.asarray(b_qkv, np.float32),
                          np.asarray(w_proj, np.float32), np.asarray(b_proj, np.float32),
                          np.asarray(gamma, np.float32), np.asarray(beta, np.float32))
    x6 = x.reshape(B, C, N)
    in_maps = [dict(x=np.ascontiguousarray(x6[BPC * i:BPC * (i + 1)]), **shared)
               for i in range(NCORES)]
    if "nc" not in _cached:
        _cached["nc"] = _build()
    res = run_bass_kernel_spmd(_cached["nc"], in_maps, list(range(NCORES)))
    out = np.empty((B, C, N), np.float32)
    for i in range(NCORES):
        out[BPC * i:BPC * (i + 1)] = res.results[i]["out"]
    return out.reshape(B, C, H, W)



# revision 11
# speedup vs baseline: 1.1153x; 1.1153x over previous
"""Trainium2 Bass kernel for an AttentionBlock (GroupNorm + MHA + proj + residual).

Shapes (hardcoded): x (16, 512, 32, 32) f32, 8 heads (ch=64), GN groups=32,
w_qkv (1536, 512), w_proj (512, 512).

Strategy: data-parallel over batch across 8 NeuronCores (2 batches/core, no
collectives). Measured-HW design rules (burst-slope microbenchmarks):
  - float32r matmuls run 1 cyc/row @2.4GHz (218ns for free=512); bf16 and
    fp8 run 1.5 cyc/row, so f32 storage + .bitcast(float32r) at the matmul
    is both fastest and most accurate. Only proj runs bf16 (SBUF pressure).
  - PSUM accumulation chains serialize (~406ns/mm) unless >=2 chains to
    different banks are interleaved instruction-by-instruction (218ns).
  - A stationary (lhsT) tile must serve >=2 consecutive matmuls, then the
    implied ldweights is fully hidden; a fresh lhsT every matmul adds 224ns.
  - ACT exp costs ~250ns + 0.87ns/elem + dep overhead: use [128,1024] tiles.
  - Engine instruction queues are in-order: an instruction whose semaphore
    wait is unsatisfied blocks everything behind it on that engine.

Structure: the attention inner loop (64 slots/batch = 8 heads x 8 k-chunks,
each: 2 score mm -> exp [128,1024] -> 2 attn@v mm, attn@v deferred 3 slots)
is ACT-bound (~1.35us/slot vs ~0.9us PE). All other work — proj+output of
batch b-1 (first, to free the x buffer), x DMA + GN stats/apply of b+1,
then v and qkv of b+1 — is emitted as atomic "filler" units interleaved
into those slots, gated by earliest-slot constraints so a filler's
semaphore wait never blocks the in-order attention stream. qk and xn are
single PERSISTENT tiles (not pool-cycled): each batch overwrites column
ranges and the framework's range-level hazards implement the ring (q/k
o-tile o of batch b+1 may be written once heads 2o,2o+1 of batch b are
done; o=3,7 spill into the first slots of the next heads block).
"""
import numpy as np
import ml_dtypes
from contextlib import ExitStack

import concourse.bass as bass
import concourse.mybir as mybir
import concourse.tile as tile
from concourse import bacc
from concourse.bass_utils import run_bass_kernel_spmd

F32 = mybir.dt.float32
F32R = mybir.dt.float32r
BF16 = mybir.dt.bfloat16
AF = mybir.ActivationFunctionType
OP = mybir.AluOpType

B, C, H, W = 16, 512, 32, 32
N = H * W            # 1024
NHEADS = 8
CH = C // NHEADS     # 64
NGROUPS = 32
GSIZE = C // NGROUPS  # 16 channels per group
EPS = 1e-5
NCORES = 8
BPC = B // NCORES    # batches per core = 2
NT = C // 128        # channel tiles per batch = 4
VW = 520             # per-n-tile v row: 8 heads x (64 ch + 1 ones col)
EXPSCALE = 1.0 / np.sqrt(float(CH))  # folded into the exp activation
SLOTS = NHEADS * 8   # 64 attention slots per batch

_cached = {}


def _build(dbg=False, reps=1, num_devices=NCORES):
    nc = bacc.Bacc("TRN2", target_bir_lowering=False, debug=False,
                   num_devices=num_devices)

    xd = nc.dram_tensor("x", [BPC, C, N], F32, kind="ExternalInput").ap()
    wqk_d = nc.dram_tensor("wqk_t", [C, 2 * C], F32R, kind="ExternalInput").ap()
    wv_d = nc.dram_tensor("wv_ext", [C, VW], F32R, kind="ExternalInput").ap()
    wp_d = nc.dram_tensor("wp_t", [C, C], BF16, kind="ExternalInput").ap()
    bqk_d = nc.dram_tensor("bqk", [128, 8], F32, kind="ExternalInput").ap()
    bv_d = nc.dram_tensor("bv_bc", [128, VW], F32, kind="ExternalInput").ap()
    bp_d = nc.dram_tensor("bp", [128, NT], F32, kind="ExternalInput").ap()
    gam_d = nc.dram_tensor("gamma_t", [128, NT], F32, kind="ExternalInput").ap()
    bet_d = nc.dram_tensor("beta_t", [128, NT], F32, kind="ExternalInput").ap()
    bd_d = nc.dram_tensor("blockdiag16", [128, 8], F32, kind="ExternalInput").ap()
    bc_d = nc.dram_tensor("bcast16", [8, 128], F32, kind="ExternalInput").ap()
    outd = nc.dram_tensor("out", [BPC, C, N], F32, kind="ExternalOutput").ap()

    with tile.TileContext(nc) as tc, ExitStack() as ctx:
        wpool = ctx.enter_context(tc.tile_pool(name="weights", bufs=1))
        xpool = ctx.enter_context(tc.tile_pool(name="x", bufs=2))
        fixpool = ctx.enter_context(tc.tile_pool(name="fix", bufs=1))
        vpool = ctx.enter_context(tc.tile_pool(name="v", bufs=2))
        hpool = ctx.enter_context(tc.tile_pool(name="h", bufs=2))
        ppool = ctx.enter_context(tc.tile_pool(name="p", bufs=4))
        opool = ctx.enter_context(tc.tile_pool(name="o", bufs=2))
        small = ctx.enter_context(tc.tile_pool(name="small", bufs=2))
        scr = ctx.enter_context(tc.tile_pool(name="scr", bufs=2))
        ps_sc = ctx.enter_context(tc.tile_pool(name="ps_sc", bufs=2, space="PSUM"))
        ps_h = ctx.enter_context(tc.tile_pool(name="ps_h", bufs=1, space="PSUM"))
        ps_fr = ctx.enter_context(tc.tile_pool(name="ps_fr", bufs=2, space="PSUM"))

        wqk_r, wv_r, wp_r = [], [], []
        for k in range(NT):
            wr = wpool.tile([128, 2 * C], F32R, tag=f"wqk{k}")
            nc.sync.dma_start(wr[:], wqk_d[128 * k:128 * (k + 1), :])
            wqk_r.append(wr)
        for k in range(NT):
            wr = wpool.tile([128, VW], F32R, tag=f"wv{k}")
            nc.sync.dma_start(wr[:], wv_d[128 * k:128 * (k + 1), :])
            wv_r.append(wr)
        for k in range(NT):
            wr = wpool.tile([128, C], BF16, tag=f"wp{k}")
            nc.sync.dma_start(wr[:], wp_d[128 * k:128 * (k + 1), :])
            wp_r.append(wr)

        bqk = wpool.tile([128, 8], F32, tag="bqk")
        nc.sync.dma_start(bqk[:], bqk_d[:])
        bv = wpool.tile([128, VW], F32, tag="bv")
        nc.sync.dma_start(bv[:], bv_d[:])
        bp = wpool.tile([128, NT], F32, tag="bp")
        nc.sync.dma_start(bp[:], bp_d[:])
        gam = wpool.tile([128, NT], F32, tag="gam")
        nc.sync.dma_start(gam[:], gam_d[:])
        bet = wpool.tile([128, NT], F32, tag="bet")
        nc.sync.dma_start(bet[:], bet_d[:])
        bd16 = wpool.tile([128, 8], F32, tag="bd16")
        nc.sync.dma_start(bd16[:], bd_d[:])
        bc16 = wpool.tile([8, 128], F32, tag="bc16")
        nc.sync.dma_start(bc16[:], bc_d[:])

        # persistent single-buffered ring tiles (range-hazard managed)
        qk_t = fixpool.tile([128, 8 * N], F32R, tag="qk", name="qk_ring")
        xn_t = fixpool.tile([128, NT * N], F32R, tag="xn", name="xn_ring")

        class St:
            pass

        # ---------------- front-end emission pieces ----------------

        def emit_load(st):
            st.x_sb = xpool.tile([128, NT * N], F32, tag="x", name="x_sb")
            for j in range(NT):
                nc.gpsimd.dma_start(st.x_sb[:, N * j:N * (j + 1)],
                                    xd[st.b, 128 * j:128 * (j + 1), :])

        def emit_stats_j(st, j):
            if j == 0:
                st.stat = small.tile([128, 8], F32, tag="stat", name="stat")
            sq = scr.tile([128, N], F32, tag="sq", bufs=2)
            nc.vector.reduce_sum(st.stat[:, j:j + 1],
                                 st.x_sb[:, N * j:N * (j + 1)],
                                 axis=mybir.AxisListType.X)
            nc.vector.scalar_tensor_tensor(
                sq[:], st.x_sb[:, N * j:N * (j + 1)], 1.0,
                st.x_sb[:, N * j:N * (j + 1)],
                op0=OP.bypass, op1=OP.mult,
                accum_out=st.stat[:, 4 + j:5 + j])

        def emit_gn_small(st):
            # group stats + Newton rsqrt + per-channel A/B (all tiny DVE ops)
            ps_st = ps_fr.tile([8, 8], F32, tag="fr")
            nc.tensor.matmul(ps_st[:], bd16[:], st.stat[:], start=True, stop=True)
            inv = 1.0 / (GSIZE * N)
            mean8 = small.tile([8, 8], F32, tag="mean8")
            nc.vector.tensor_scalar_mul(mean8[:, 0:4], ps_st[:, 0:4], inv)
            ex2 = small.tile([8, 4], F32, tag="ex2")
            nc.vector.tensor_scalar_mul(ex2[:], ps_st[:, 4:8], inv)
            m2 = small.tile([8, 4], F32, tag="m2")
            nc.vector.tensor_mul(m2[:], mean8[:, 0:4], mean8[:, 0:4])
            veps = small.tile([8, 4], F32, tag="veps")
            nc.vector.scalar_tensor_tensor(veps[:], ex2[:], EPS, m2[:],
                                           op0=OP.add, op1=OP.subtract)
            r_cur = small.tile([8, 4], F32, tag="r0")
            nc.vector.tensor_scalar(r_cur[:], veps[:], -0.5, 1.5,
                                    op0=OP.mult, op1=OP.add)
            for it in range(3):
                t1 = small.tile([8, 4], F32, tag=f"nt1_{it}")
                nc.vector.tensor_mul(t1[:], r_cur[:], r_cur[:])
                t2 = small.tile([8, 4], F32, tag=f"nt2_{it}")
                nc.vector.scalar_tensor_tensor(t2[:], t1[:], -0.5, veps[:],
                                               op0=OP.mult, op1=OP.mult)
                t3 = small.tile([8, 4], F32, tag=f"nt3_{it}")
                nc.vector.tensor_scalar_add(t3[:], t2[:], 1.5)
                r_nxt = small.tile([8, 4], F32, tag=f"nr_{it}")
                nc.vector.tensor_mul(r_nxt[:], r_cur[:], t3[:])
                r_cur = r_nxt
            nc.vector.tensor_copy(mean8[:, 4:8], r_cur[:])
            ps_bc = ps_fr.tile([128, 8], F32, tag="fr")
            nc.tensor.matmul(ps_bc[:], bc16[:], mean8[:], start=True, stop=True)
            st.A_ch = small.tile([128, NT], F32, tag="A_ch", name="A_ch")
            nc.vector.tensor_mul(st.A_ch[:], gam[:], ps_bc[:, 4:8])
            tB = small.tile([128, NT], F32, tag="tB")
            nc.vector.tensor_mul(tB[:], ps_bc[:, 0:4], st.A_ch[:])
            st.B_ch = small.tile([128, NT], F32, tag="B_ch", name="B_ch")
            nc.vector.scalar_tensor_tensor(st.B_ch[:], tB[:], -1.0, bet[:],
                                           op0=OP.mult, op1=OP.add)

        def emit_apply_j(st, j):
            nc.vector.tensor_scalar(xn_t[:, N * j:N * (j + 1)],
                                    st.x_sb[:, N * j:N * (j + 1)],
                                    st.A_ch[:, j:j + 1], st.B_ch[:, j:j + 1],
                                    op0=OP.mult, op1=OP.add)

        def emit_v_nt(st, ntile):
            # v^T n-tile: out [128 n, 520 c] (8 heads x 64 ch + zero-weight
            # ones cols whose 1.0 comes from the bias), 4 accumulated k-tiles
            # as two interleaved chains (free 512 + free 8)
            if ntile == 0:
                st.vv = vpool.tile([128, 8 * VW], F32R, tag="vv", name="vv")
            pva = ps_fr.tile([128, 512], F32, tag="fr", name=f"pva{ntile}")
            pvb = ps_fr.tile([128, 8], F32, tag="fr", name=f"pvb{ntile}")
            for k in range(NT):
                xsl = xn_t[:, N * k + 128 * ntile:N * k + 128 * (ntile + 1)]
                nc.tensor.matmul(pva[:], xsl, wv_r[k][:, 0:512],
                                 start=(k == 0), stop=(k == NT - 1))
                nc.tensor.matmul(pvb[:], xsl, wv_r[k][:, 512:VW],
                                 start=(k == 0), stop=(k == NT - 1))
            nc.vector.scalar_tensor_tensor(
                st.vv[:, VW * ntile:VW * ntile + 512], pva[:], 1.0,
                bv[:, 0:512], op0=OP.bypass, op1=OP.add)
            nc.vector.scalar_tensor_tensor(
                st.vv[:, VW * ntile + 512:VW * (ntile + 1)], pvb[:], 1.0,
                bv[:, 512:VW], op0=OP.bypass, op1=OP.add)

        def emit_qkv_o(st, o):
            # q/k o-tile: out [128 oc, 1024 n] via 2 interleaved nh chains
            pq = [ps_fr.tile([128, 512], F32, tag="fr", name=f"pq{o}_{nh}")
                  for nh in range(2)]
            for k in range(NT):
                for nh in range(2):
                    nc.tensor.matmul(
                        pq[nh][:],
                        (wqk_r[k][:, 128 * o:128 * (o + 1)]),
                        (xn_t[:, N * k + 512 * nh:N * k + 512 * (nh + 1)]),
                        start=(k == 0), stop=(k == NT - 1))
            for nh in range(2):
                nc.vector.tensor_scalar_add(
                    qk_t[:, N * o + 512 * nh:N * o + 512 * (nh + 1)],
                    pq[nh][:], bqk[:, o:o + 1])

        def emit_proj_o(st, o):
            pp = [ps_fr.tile([128, 512], F32, tag="fr", name=f"pp{o}_{nh}")
                  for nh in range(2)]
            for k in range(NT):
                for nh in range(2):
                    nc.tensor.matmul(
                        pp[nh][:],
                        wp_r[k][:, 128 * o:128 * (o + 1)],
                        st.hall[:, N * k + 512 * nh:N * k + 512 * (nh + 1)],
                        start=(k == 0), stop=(k == NT - 1))
            for nh in range(2):
                ot = opool.tile([128, 512], F32, tag="ot")
                nc.vector.scalar_tensor_tensor(
                    ot[:], pp[nh][:], bp[:, o:o + 1],
                    st.x_sb[:, N * o + 512 * nh:N * o + 512 * (nh + 1)],
                    op0=OP.add, op1=OP.add)
                nc.sync.dma_start(
                    outd[st.b, 128 * o:128 * (o + 1), 512 * nh:512 * (nh + 1)],
                    ot[:])

        def emit_front_straight(st):
            emit_load(st)
            for j in range(NT):
                emit_stats_j(st, j)
            emit_gn_small(st)
            for j in range(NT):
                emit_apply_j(st, j)
            for ntile in range(8):
                emit_v_nt(st, ntile)
            for o in range(8):
                emit_qkv_o(st, o)

        # ---------------- filler units ----------------
        # unit = (min_slot_within_this_heads_block, weight, closure)

        def proj_units(st):
            return [(1 + 2 * o, 8.0, lambda o=o: emit_proj_o(st, o))
                    for o in range(NT)]

        def front_units(st):
            """Units for batch st's front-end, run under heads(prev).
            Returns (units, late_units); late_units go into the NEXT heads
            block (qk o-tiles 3 and 7 are still being read until the last
            head of heads(prev))."""
            u = [(9, 0.0, lambda: emit_load(st))]
            for j in range(NT):
                u.append((14 + j, 0.7, lambda j=j: emit_stats_j(st, j)))
            u.append((19, 1.0, lambda: emit_gn_small(st)))
            for j in range(NT):
                u.append((21 + j, 0.7, lambda j=j: emit_apply_j(st, j)))
            for ntile in range(8):
                u.append((26 + 2 * ntile, 4.0,
                          lambda n_=ntile: emit_v_nt(st, n_)))
            # qk ring gates: o-tile o writable after heads 2o,2o+1 done
            for o in range(3):
                for oo in (o, o + 4):
                    u.append((max((2 * o + 2) * 8, 27 + 2 * o), 8.0,
                              lambda oo=oo: emit_qkv_o(st, oo)))
            late = [(1 + 2 * i, 8.0, lambda oo=oo: emit_qkv_o(st, oo))
                    for i, oo in enumerate((3, 7))]
            return u, late

        # ---------------- attention (the slot loop) ----------------

        def emit_heads(st, fillers):
            DEFER = 3
            fq = sorted(fillers, key=lambda t: t[0])
            fi = 0
            st.hall = hpool.tile([128, NT * N], BF16, tag="hall", name="hall")
            pend = []  # (head, j, p_tile) awaiting attn@v
            phs = {}

            def emit_av(h, j, p_t):
                if j == 0:
                    phs[h] = ps_h.tile([65, N], F32, tag="hacc",
                                       name=f"phs{h}")
                for th in range(2):
                    nc.tensor.matmul(
                        phs[h][:, 512 * th:512 * (th + 1)],
                        (st.vv[:, VW * j + 65 * h:VW * j + 65 * h + 65]),
                        (p_t[:, 512 * th:512 * (th + 1)]),
                        start=(j == 0), stop=(j == 7))

            def emit_norm(h):
                ph = phs.pop(h)
                rec = small.tile([1, N], F32, tag="rec")
                nc.vector.reciprocal(rec[:], ph[64:65, :])
                rb = scr.tile([64, N], F32, tag="rb", bufs=1)
                nc.gpsimd.partition_broadcast(rb[:], rec[:])
                nc.vector.tensor_mul(
                    st.hall[64 * (h % 2):64 * (h % 2) + 64,
                            N * (h // 2):N * (h // 2 + 1)],
                    ph[0:64, :], rb[:])

            for h in range(NHEADS):
                pb = 64 * (h % 2)
                q_base = N * (h // 2)
                k_base = 4 * N + N * (h // 2)
                for j in range(8):
                    slot = 8 * h + j
                    sc = ps_sc.tile([128, N], F32, tag="sc")
                    for th in range(2):
                        nc.tensor.matmul(
                            sc[:, 512 * th:512 * (th + 1)],
                            (qk_t[pb:pb + 64, k_base + 128 * j:
                                    k_base + 128 * (j + 1)]),
                            (qk_t[pb:pb + 64, q_base + 512 * th:
                                    q_base + 512 * (th + 1)]),
                            start=True, stop=True,
                            tile_position=(pb, 0))
                    p_t = ppool.tile([128, N], F32R, tag="p")
                    nc.scalar.activation(p_t[:], sc[:], AF.Exp, scale=EXPSCALE)
                    pend.append((h, j, p_t))
                    while pend and (8 * pend[0][0] + pend[0][1] + DEFER
                                    <= slot):
                        ah, aj, ap_t = pend.pop(0)
                        emit_av(ah, aj, ap_t)
                        if aj == 7:
                            emit_norm(ah)
                    budget = 2.5
                    while fi < len(fq) and budget > 0:
                        ms, wgt, fn = fq[fi]
                        if ms > slot:
                            break
                        fn()
                        budget -= wgt
                        fi += 1
            while pend:
                ah, aj, ap_t = pend.pop(0)
                emit_av(ah, aj, ap_t)
                if aj == 7:
                    emit_norm(ah)
            while fi < len(fq):
                fq[fi][2]()
                fi += 1

        # ---------------- batch pipeline ----------------
        bs = [b for _ in range(reps) for b in range(BPC)]
        sts = []
        st0 = St()
        st0.b = bs[0]
        emit_front_straight(st0)  # prologue
        sts.append(st0)
        carry = []  # late units from front(b+1) for the next heads block
        for idx in range(len(bs)):
            st = sts[idx]
            fillers = list(carry)
            carry = []
            if idx >= 1:
                fillers += proj_units(sts[idx - 1])
            if idx + 1 < len(bs):
                nxt = St()
                nxt.b = bs[idx + 1]
                sts.append(nxt)
                u, late = front_units(nxt)
                fillers += u
                carry = late
            emit_heads(st, fillers)
        for fn in [t[2] for t in carry]:
            fn()
        last = sts[-1]
        for o in range(NT):
            emit_proj_o(last, o)

    nc.compile()
    return nc


def _prep_shared(w_qkv, b_qkv, w_proj, b_proj, gamma, beta):
    r = np.arange(3 * C).reshape(NHEADS, 3, CH)
    idx_q, idx_k, idx_v = r[:, 0].ravel(), r[:, 1].ravel(), r[:, 2].ravel()
    wqk_t = np.ascontiguousarray(
        np.concatenate([w_qkv[idx_q], w_qkv[idx_k]], axis=0).T).astype(np.float32)
    bqk_full = np.concatenate([b_qkv[idx_q], b_qkv[idx_k]])
    bqk = np.ascontiguousarray(bqk_full.reshape(8, 128).T).astype(np.float32)

    wv = w_qkv[idx_v]
    bv_src = b_qkv[idx_v]
    wv_ext = np.zeros((C, VW), np.float32)
    bv_ext = np.zeros((VW,), np.float32)
    for h in range(NHEADS):
        wv_ext[:, 65 * h:65 * h + CH] = wv[CH * h:CH * (h + 1), :].T
        bv_ext[65 * h:65 * h + CH] = bv_src[CH * h:CH * (h + 1)]
        bv_ext[65 * h + CH] = 1.0
    bv_bc = np.ascontiguousarray(np.broadcast_to(bv_ext, (128, VW))).astype(np.float32)

    wp_t = np.ascontiguousarray(w_proj.T).astype(ml_dtypes.bfloat16)
    bp = np.ascontiguousarray(b_proj.reshape(NT, 128).T).astype(np.float32)
    gamma_t = np.ascontiguousarray(gamma.reshape(NT, 128).T).astype(np.float32)
    beta_t = np.ascontiguousarray(beta.reshape(NT, 128).T).astype(np.float32)
    blockdiag16 = np.kron(np.eye(8, dtype=np.float32), np.ones((GSIZE, 1), np.float32))
    bcast16 = np.ascontiguousarray(blockdiag16.T)
    return dict(wqk_t=wqk_t, bqk=bqk, wv_ext=wv_ext, bv_bc=bv_bc, wp_t=wp_t,
                bp=bp, gamma_t=gamma_t, beta_t=beta_t,
                blockdiag16=blockdiag16, bcast16=bcast16)


def kernel(x, gamma, beta, w_qkv, b_qkv, w_proj, b_proj):
    x = np.asarray(x, dtype=np.float32)
    shared = _prep_shared(np.asarray(w_qkv, np.float32), np.asarray(b_qkv, np.float32),
                          np.asarray(w_proj, np.float32), np.asarray(b_proj, np.float32),
                          np.asarray(gamma, np.float32), np.asarray(beta, np.float32))
    x6 = x.reshape(B, C, N)
    in_maps = [dict(x=np.ascontiguousarray(x6[BPC * i:BPC * (i + 1)]), **shared)
               for i in range(NCORES)]
    if "nc" not in _cached:
        _cached["nc"] = _build()
    res = run_bass_kernel_spmd(_cached["nc"], in_maps, list(range(NCORES)))
    out = np.empty((B, C, N), np.float32)
    for i in range(NCORES):
        out[BPC * i:BPC * (i + 1)] = res.results[i]["out"]
    return out.reshape(B, C, H, W)


# revision 19
# speedup vs baseline: 1.1251x; 1.0087x over previous
"""Trainium2 Bass kernel for an AttentionBlock (GroupNorm + MHA + proj + residual).

Shapes (hardcoded): x (16, 512, 32, 32) f32, 8 heads (ch=64), GN groups=32,
w_qkv (1536, 512), w_proj (512, 512).

Strategy: data-parallel over batch across 8 NeuronCores (2 batches/core, no
collectives). Measured-HW design rules (burst-slope microbenchmarks):
  - float32r matmuls run 1 cyc/row @2.4GHz (218ns for free=512); bf16 and
    fp8 run 1.5 cyc/row, so f32 storage + .bitcast(float32r) at the matmul
    is both fastest and most accurate. Only proj runs bf16 (SBUF pressure).
  - PSUM accumulation chains serialize (~406ns/mm) unless >=2 chains to
    different banks are interleaved instruction-by-instruction (218ns).
  - A stationary (lhsT) tile must serve >=2 consecutive matmuls, then the
    implied ldweights is fully hidden; a fresh lhsT every matmul adds 224ns.
  - ACT exp costs ~250ns + 0.87ns/elem + dep overhead: use [128,1024] tiles.
  - Engine instruction queues are in-order: an instruction whose semaphore
    wait is unsatisfied blocks everything behind it on that engine.

Structure: the attention inner loop (64 slots/batch = 8 heads x 8 k-chunks,
each: 2 score mm -> exp [128,1024] -> 2 attn@v mm, attn@v deferred 3 slots)
is ACT-bound (~1.35us/slot vs ~0.9us PE). All other work — proj+output of
batch b-1 (first, to free the x buffer), x DMA + GN stats/apply of b+1,
then v and qkv of b+1 — is emitted as atomic "filler" units interleaved
into those slots, gated by earliest-slot constraints so a filler's
semaphore wait never blocks the in-order attention stream. qk and xn are
single PERSISTENT tiles (not pool-cycled): each batch overwrites column
ranges and the framework's range-level hazards implement the ring (q/k
o-tile o of batch b+1 may be written once heads 2o,2o+1 of batch b are
done; o=3,7 spill into the first slots of the next heads block).
"""
import numpy as np
import ml_dtypes
from contextlib import ExitStack

import concourse.bass as bass
import concourse.mybir as mybir
import concourse.tile as tile
from concourse import bacc
from concourse.bass_utils import run_bass_kernel_spmd

F32 = mybir.dt.float32
F32R = mybir.dt.float32r
BF16 = mybir.dt.bfloat16
AF = mybir.ActivationFunctionType
OP = mybir.AluOpType

B, C, H, W = 16, 512, 32, 32
N = H * W            # 1024
NHEADS = 8
CH = C // NHEADS     # 64
NGROUPS = 32
GSIZE = C // NGROUPS  # 16 channels per group
EPS = 1e-5
NCORES = 8
BPC = B // NCORES    # batches per core = 2
NT = C // 128        # channel tiles per batch = 4
VW = 520             # per-n-tile v row: 8 heads x (64 ch + 1 ones col)
EXPSCALE = 1.0 / np.sqrt(float(CH))  # folded into the exp activation
SLOTS = NHEADS * 8   # 64 attention slots per batch

_cached = {}


def _build(dbg=False, reps=1, num_devices=NCORES, interleave=True,
           defer=4, p_bufs=5, mode='full', attnonly=False, frontonly=False):
    nc = bacc.Bacc("TRN2", target_bir_lowering=False, debug=False,
                   num_devices=num_devices)

    xd = nc.dram_tensor("x", [BPC, C, N], F32, kind="ExternalInput").ap()
    wqk_d = nc.dram_tensor("wqk_t", [C, 2 * C], F32R, kind="ExternalInput").ap()
    wv_d = nc.dram_tensor("wv_ext", [C, VW], F32R, kind="ExternalInput").ap()
    wp_d = nc.dram_tensor("wp_t", [C, C], BF16, kind="ExternalInput").ap()
    bqk_d = nc.dram_tensor("bqk", [128, 8], F32, kind="ExternalInput").ap()
    bv_d = nc.dram_tensor("bv_bc", [128, VW], F32, kind="ExternalInput").ap()
    bp_d = nc.dram_tensor("bp", [128, NT], F32, kind="ExternalInput").ap()
    gam_d = nc.dram_tensor("gamma_t", [128, NT], F32, kind="ExternalInput").ap()
    bet_d = nc.dram_tensor("beta_t", [128, NT], F32, kind="ExternalInput").ap()
    bd_d = nc.dram_tensor("blockdiag16", [128, 8], F32, kind="ExternalInput").ap()
    bc_d = nc.dram_tensor("bcast16", [8, 128], F32, kind="ExternalInput").ap()
    outd = nc.dram_tensor("out", [BPC, C, N], F32, kind="ExternalOutput").ap()

    with tile.TileContext(nc) as tc, ExitStack() as ctx:
        wpool = ctx.enter_context(tc.tile_pool(name="weights", bufs=1))
        xpool = ctx.enter_context(tc.tile_pool(name="x", bufs=2))
        fixpool = ctx.enter_context(tc.tile_pool(name="fix", bufs=1))
        vpool = ctx.enter_context(tc.tile_pool(name="v", bufs=2))
        hpool = ctx.enter_context(tc.tile_pool(name="h", bufs=2))
        ppool = ctx.enter_context(tc.tile_pool(name="p", bufs=p_bufs))
        opool = ctx.enter_context(tc.tile_pool(name="o", bufs=2))
        small = ctx.enter_context(tc.tile_pool(name="small", bufs=2))
        scr = ctx.enter_context(tc.tile_pool(name="scr", bufs=2))
        ps_sc = ctx.enter_context(tc.tile_pool(name="ps_sc", bufs=2, space="PSUM"))
        ps_h = ctx.enter_context(tc.tile_pool(name="ps_h", bufs=1, space="PSUM"))
        ps_fr = ctx.enter_context(tc.tile_pool(name="ps_fr", bufs=2, space="PSUM"))

        wqk_r, wv_r, wp_r = [], [], []
        for k in range(NT):
            wr = wpool.tile([128, 2 * C], F32R, tag=f"wqk{k}")
            nc.sync.dma_start(wr[:], wqk_d[128 * k:128 * (k + 1), :])
            wqk_r.append(wr)
        for k in range(NT):
            wr = wpool.tile([128, VW], F32R, tag=f"wv{k}")
            nc.sync.dma_start(wr[:], wv_d[128 * k:128 * (k + 1), :])
            wv_r.append(wr)
        for k in range(NT):
            wr = wpool.tile([128, C], BF16, tag=f"wp{k}")
            nc.sync.dma_start(wr[:], wp_d[128 * k:128 * (k + 1), :])
            wp_r.append(wr)

        bqk = wpool.tile([128, 8], F32, tag="bqk")
        nc.sync.dma_start(bqk[:], bqk_d[:])
        bv = wpool.tile([128, VW], F32, tag="bv")
        nc.sync.dma_start(bv[:], bv_d[:])
        bp = wpool.tile([128, NT], F32, tag="bp")
        nc.sync.dma_start(bp[:], bp_d[:])
        gam = wpool.tile([128, NT], F32, tag="gam")
        nc.sync.dma_start(gam[:], gam_d[:])
        bet = wpool.tile([128, NT], F32, tag="bet")
        nc.sync.dma_start(bet[:], bet_d[:])
        bd16 = wpool.tile([128, 8], F32, tag="bd16")
        nc.sync.dma_start(bd16[:], bd_d[:])
        bc16 = wpool.tile([8, 128], F32, tag="bc16")
        nc.sync.dma_start(bc16[:], bc_d[:])

        # persistent single-buffered ring tiles (range-hazard managed)
        qk_t = fixpool.tile([128, 8 * N], F32R, tag="qk", name="qk_ring")
        xn_t = fixpool.tile([128, NT * N], F32R, tag="xn", name="xn_ring")
        pconst = None
        if mode in ("noexp", "constp", "noexp_nonorm"):
            pconst = fixpool.tile([128, N], F32R, tag="pconst", name="pconst")
            nc.gpsimd.dma_start(pconst[:], xd[0, 0:128, :])

        class St:
            pass

        # ---------------- front-end emission pieces ----------------

        def emit_load(st):
            st.x_sb = xpool.tile([128, NT * N], F32, tag="x", name="x_sb")
            for j in range(NT):
                nc.gpsimd.dma_start(st.x_sb[:, N * j:N * (j + 1)],
                                    xd[st.b, 128 * j:128 * (j + 1), :])

        def emit_stats_j(st, j):
            if j == 0:
                st.stat = small.tile([128, 8], F32, tag="stat", name="stat")
            sq = scr.tile([128, N], F32, tag="sq", bufs=2)
            nc.vector.reduce_sum(st.stat[:, j:j + 1],
                                 st.x_sb[:, N * j:N * (j + 1)],
                                 axis=mybir.AxisListType.X)
            nc.vector.scalar_tensor_tensor(
                sq[:], st.x_sb[:, N * j:N * (j + 1)], 1.0,
                st.x_sb[:, N * j:N * (j + 1)],
                op0=OP.bypass, op1=OP.mult,
                accum_out=st.stat[:, 4 + j:5 + j])

        def emit_gn_small(st):
            # group stats + Newton rsqrt + per-channel A/B (all tiny DVE ops)
            ps_st = ps_fr.tile([8, 8], F32, tag="fr")
            nc.tensor.matmul(ps_st[:], bd16[:], st.stat[:], start=True, stop=True)
            inv = 1.0 / (GSIZE * N)
            mean8 = small.tile([8, 8], F32, tag="mean8")
            nc.vector.tensor_scalar_mul(mean8[:, 0:4], ps_st[:, 0:4], inv)
            ex2 = small.tile([8, 4], F32, tag="ex2")
            nc.vector.tensor_scalar_mul(ex2[:], ps_st[:, 4:8], inv)
            m2 = small.tile([8, 4], F32, tag="m2")
            nc.vector.tensor_mul(m2[:], mean8[:, 0:4], mean8[:, 0:4])
            veps = small.tile([8, 4], F32, tag="veps")
            nc.vector.scalar_tensor_tensor(veps[:], ex2[:], EPS, m2[:],
                                           op0=OP.add, op1=OP.subtract)
            r_cur = small.tile([8, 4], F32, tag="r0")
            nc.vector.tensor_scalar(r_cur[:], veps[:], -0.5, 1.5,
                                    op0=OP.mult, op1=OP.add)
            for it in range(3):
                t1 = small.tile([8, 4], F32, tag=f"nt1_{it}")
                nc.vector.tensor_mul(t1[:], r_cur[:], r_cur[:])
                t2 = small.tile([8, 4], F32, tag=f"nt2_{it}")
                nc.vector.scalar_tensor_tensor(t2[:], t1[:], -0.5, veps[:],
                                               op0=OP.mult, op1=OP.mult)
                t3 = small.tile([8, 4], F32, tag=f"nt3_{it}")
                nc.vector.tensor_scalar_add(t3[:], t2[:], 1.5)
                r_nxt = small.tile([8, 4], F32, tag=f"nr_{it}")
                nc.vector.tensor_mul(r_nxt[:], r_cur[:], t3[:])
                r_cur = r_nxt
            nc.vector.tensor_copy(mean8[:, 4:8], r_cur[:])
            ps_bc = ps_fr.tile([128, 8], F32, tag="fr")
            nc.tensor.matmul(ps_bc[:], bc16[:], mean8[:], start=True, stop=True)
            st.A_ch = small.tile([128, NT], F32, tag="A_ch", name="A_ch")
            nc.vector.tensor_mul(st.A_ch[:], gam[:], ps_bc[:, 4:8])
            tB = small.tile([128, NT], F32, tag="tB")
            nc.vector.tensor_mul(tB[:], ps_bc[:, 0:4], st.A_ch[:])
            st.B_ch = small.tile([128, NT], F32, tag="B_ch", name="B_ch")
            nc.vector.scalar_tensor_tensor(st.B_ch[:], tB[:], -1.0, bet[:],
                                           op0=OP.mult, op1=OP.add)

        def emit_apply_j(st, j):
            nc.vector.tensor_scalar(xn_t[:, N * j:N * (j + 1)],
                                    st.x_sb[:, N * j:N * (j + 1)],
                                    st.A_ch[:, j:j + 1], st.B_ch[:, j:j + 1],
                                    op0=OP.mult, op1=OP.add)

        def emit_v_nt(st, ntile):
            # v^T n-tile: out [128 n, 520 c] (8 heads x 64 ch + zero-weight
            # ones cols whose 1.0 comes from the bias), 4 accumulated k-tiles
            # as two interleaved chains (free 512 + free 8)
            if ntile == 0:
                st.vv = vpool.tile([128, 8 * VW], F32R, tag="vv", name="vv")
            pva = ps_fr.tile([128, 512], F32, tag="fr", name=f"pva{ntile}")
            pvb = ps_fr.tile([128, 8], F32, tag="fr", name=f"pvb{ntile}")
            for k in range(NT):
                xsl = xn_t[:, N * k + 128 * ntile:N * k + 128 * (ntile + 1)]
                nc.tensor.matmul(pva[:], xsl, wv_r[k][:, 0:512],
                                 start=(k == 0), stop=(k == NT - 1))
                nc.tensor.matmul(pvb[:], xsl, wv_r[k][:, 512:VW],
                                 start=(k == 0), stop=(k == NT - 1))
            nc.vector.scalar_tensor_tensor(
                st.vv[:, VW * ntile:VW * ntile + 512], pva[:], 1.0,
                bv[:, 0:512], op0=OP.bypass, op1=OP.add)
            nc.vector.scalar_tensor_tensor(
                st.vv[:, VW * ntile + 512:VW * (ntile + 1)], pvb[:], 1.0,
                bv[:, 512:VW], op0=OP.bypass, op1=OP.add)

        def emit_qkv_o(st, o):
            # q/k o-tile: out [128 oc, 1024 n] via 2 interleaved nh chains
            pq = [ps_fr.tile([128, 512], F32, tag="fr", name=f"pq{o}_{nh}")
                  for nh in range(2)]
            for k in range(NT):
                for nh in range(2):
                    nc.tensor.matmul(
                        pq[nh][:],
                        (wqk_r[k][:, 128 * o:128 * (o + 1)]),
                        (xn_t[:, N * k + 512 * nh:N * k + 512 * (nh + 1)]),
                        start=(k == 0), stop=(k == NT - 1))
            for nh in range(2):
                nc.vector.tensor_scalar_add(
                    qk_t[:, N * o + 512 * nh:N * o + 512 * (nh + 1)],
                    pq[nh][:], bqk[:, o:o + 1])

        def emit_proj_o(st, o):
            pp = [ps_fr.tile([128, 512], F32, tag="fr", name=f"pp{o}_{nh}")
                  for nh in range(2)]
            for k in range(NT):
                for nh in range(2):
                    nc.tensor.matmul(
                        pp[nh][:],
                        wp_r[k][:, 128 * o:128 * (o + 1)],
                        st.hall[:, N * k + 512 * nh:N * k + 512 * (nh + 1)],
                        start=(k == 0), stop=(k == NT - 1))
            for nh in range(2):
                ot = opool.tile([128, 512], F32, tag="ot")
                nc.vector.scalar_tensor_tensor(
                    ot[:], pp[nh][:], bp[:, o:o + 1],
                    st.x_sb[:, N * o + 512 * nh:N * o + 512 * (nh + 1)],
                    op0=OP.add, op1=OP.add)
                nc.sync.dma_start(
                    outd[st.b, 128 * o:128 * (o + 1), 512 * nh:512 * (nh + 1)],
                    ot[:])

        def emit_front_straight(st):
            emit_load(st)
            for j in range(NT):
                emit_stats_j(st, j)
            emit_gn_small(st)
            for j in range(NT):
                emit_apply_j(st, j)
            for ntile in range(8):
                emit_v_nt(st, ntile)
            for o in range(8):
                emit_qkv_o(st, o)

        # ---------------- filler units ----------------
        # unit = (min_slot_within_this_heads_block, weight, closure)

        def proj_units(st):
            return [(1 + 2 * o, 8.0, lambda o=o: emit_proj_o(st, o))
                    for o in range(NT)]

        def front_units(st):
            """Units for batch st's front-end, run under heads(prev).
            Returns (units, late_units); late_units go into the NEXT heads
            block (qk o-tiles 3 and 7 are still being read until the last
            head of heads(prev))."""
            u = [(9, 0.0, lambda: emit_load(st))]
            for j in range(NT):
                u.append((14 + j, 0.7, lambda j=j: emit_stats_j(st, j)))
            u.append((19, 1.0, lambda: emit_gn_small(st)))
            for j in range(NT):
                u.append((21 + j, 0.7, lambda j=j: emit_apply_j(st, j)))
            for ntile in range(8):
                u.append((26 + 2 * ntile, 4.0,
                          lambda n_=ntile: emit_v_nt(st, n_)))
            # qk ring gates: o-tile o writable after heads 2o,2o+1 done
            for o in range(3):
                for oo in (o, o + 4):
                    u.append((max((2 * o + 2) * 8, 27 + 2 * o), 8.0,
                              lambda oo=oo: emit_qkv_o(st, oo)))
            late = [(1 + 2 * i, 8.0, lambda oo=oo: emit_qkv_o(st, oo))
                    for i, oo in enumerate((3, 7))]
            return u, late

        # ---------------- attention (the slot loop) ----------------

        def emit_heads(st, fillers):
            DEFER = defer
            skipnorm = mode in ("nonorm", "noexp_nonorm")
            fq = sorted(fillers, key=lambda t: t[0])
            fi = 0
            st.hall = hpool.tile([128, NT * N], BF16, tag="hall", name="hall")
            pend = []  # (head, j, p_tile) awaiting attn@v
            phs = {}

            def emit_av(h, j, p_t):
                if j == 0:
                    phs[h] = ps_h.tile([65, N], F32, tag="hacc",
                                       name=f"phs{h}")
                for th in range(2):
                    nc.tensor.matmul(
                        phs[h][:, 512 * th:512 * (th + 1)],
                        (st.vv[:, VW * j + 65 * h:VW * j + 65 * h + 65]),
                        (p_t[:, 512 * th:512 * (th + 1)]),
                        start=(j == 0), stop=(j == 7))

            def emit_norm(h):
                ph = phs.pop(h)
                if skipnorm:
                    nc.vector.tensor_copy(
                        st.hall[64 * (h % 2):64 * (h % 2) + 64,
                                N * (h // 2):N * (h // 2 + 1)],
                        ph[0:64, :])
                    return
                rec = small.tile([1, N], F32, tag="rec")
                nc.vector.reciprocal(rec[:], ph[64:65, :])
                rb = scr.tile([64, N], F32, tag="rb", bufs=1)
                nc.gpsimd.partition_broadcast(rb[:], rec[:])
                nc.vector.tensor_mul(
                    st.hall[64 * (h % 2):64 * (h % 2) + 64,
                            N * (h // 2):N * (h // 2 + 1)],
                    ph[0:64, :], rb[:])

            for h in range(NHEADS):
                pb = 64 * (h % 2)
                q_base = N * (h // 2)
                k_base = 4 * N + N * (h // 2)
                for j2 in range(0, 8, 2):
                    # run of 4 score matmuls (two slots' worth): fewer PE
                    # array tile-config switches than alternating sc/av
                    for j in (j2, j2 + 1):
                        sc = ps_sc.tile([128, N], F32, tag="sc")
                        for th in range(2):
                            nc.tensor.matmul(
                                sc[:, 512 * th:512 * (th + 1)],
                                (qk_t[pb:pb + 64, k_base + 128 * j:
                                        k_base + 128 * (j + 1)]),
                                (qk_t[pb:pb + 64, q_base + 512 * th:
                                        q_base + 512 * (th + 1)]),
                                start=True, stop=True,
                                tile_position=(pb, 0))
                        if mode not in ("noexp", "noexp_nonorm"):
                            p_t = ppool.tile([128, N], F32R, tag="p")
                            nc.scalar.activation(p_t[:], sc[:], AF.Exp,
                                                 scale=EXPSCALE)
                        if mode in ("noexp", "constp", "noexp_nonorm"):
                            pend.append((h, j, pconst))
                        else:
                            pend.append((h, j, p_t))
                    slot = 8 * h + j2 + 1
                    while pend and (8 * pend[0][0] + pend[0][1] + DEFER
                                    <= slot):
                        ah, aj, ap_t = pend.pop(0)
                        emit_av(ah, aj, ap_t)
                        if aj == 7:
                            emit_norm(ah)
                    budget = 5.0
                    while fi < len(fq) and budget > 0:
                        ms, wgt, fn = fq[fi]
                        if ms > slot:
                            break
                        fn()
                        budget -= wgt
                        fi += 1
            while pend:
                ah, aj, ap_t = pend.pop(0)
                emit_av(ah, aj, ap_t)
                if aj == 7:
                    emit_norm(ah)
            while fi < len(fq):
                fq[fi][2]()
                fi += 1

        # ---------------- batch pipeline ----------------
        bs = [b for _ in range(reps) for b in range(BPC)]
        sts = []
        st0 = St()
        st0.b = bs[0]
        emit_front_straight(st0)  # prologue
        sts.append(st0)
        carry = []  # late units from front(b+1) for the next heads block
        if attnonly:
            for idx in range(len(bs)):
                emit_heads(st0, [])
            nc.compile()
            return nc
        if frontonly:
            for idx in range(1, len(bs)):
                sti = St()
                sti.b = bs[idx]
                emit_front_straight(sti)
            nc.compile()
            return nc
        for idx in range(len(bs)):
            st = sts[idx]
            fillers = list(carry)
            carry = []
            if idx >= 1:
                fillers += proj_units(sts[idx - 1])
            if idx + 1 < len(bs):
                nxt = St()
                nxt.b = bs[idx + 1]
                sts.append(nxt)
                u, late = front_units(nxt)
                fillers += u
                carry = late
            if interleave:
                emit_heads(st, fillers)
            else:
                emit_heads(st, [])
                for _ms, _w, fn in sorted(fillers + carry,
                                          key=lambda t: t[0]):
                    fn()
                carry = []
        for fn in [t[2] for t in carry]:
            fn()
        last = sts[-1]
        for o in range(NT):
            emit_proj_o(last, o)

    nc.compile()
    return nc


def _prep_shared(w_qkv, b_qkv, w_proj, b_proj, gamma, beta):
    r = np.arange(3 * C).reshape(NHEADS, 3, CH)
    idx_q, idx_k, idx_v = r[:, 0].ravel(), r[:, 1].ravel(), r[:, 2].ravel()
    wqk_t = np.ascontiguousarray(
        np.concatenate([w_qkv[idx_q], w_qkv[idx_k]], axis=0).T).astype(np.float32)
    bqk_full = np.concatenate([b_qkv[idx_q], b_qkv[idx_k]])
    bqk = np.ascontiguousarray(bqk_full.reshape(8, 128).T).astype(np.float32)

    wv = w_qkv[idx_v]
    bv_src = b_qkv[idx_v]
    wv_ext = np.zeros((C, VW), np.float32)
    bv_ext = np.zeros((VW,), np.float32)
    for h in range(NHEADS):
        wv_ext[:, 65 * h:65 * h + CH] = wv[CH * h:CH * (h + 1), :].T
        bv_ext[65 * h:65 * h + CH] = bv_src[CH * h:CH * (h + 1)]
        bv_ext[65 * h + CH] = 1.0
    bv_bc = np.ascontiguousarray(np.broadcast_to(bv_ext, (128, VW))).astype(np.float32)

    wp_t = np.ascontiguousarray(w_proj.T).astype(ml_dtypes.bfloat16)
    bp = np.ascontiguousarray(b_proj.reshape(NT, 128).T).astype(np.float32)
    gamma_t = np.ascontiguousarray(gamma.reshape(NT, 128).T).astype(np.float32)
    beta_t = np.ascontiguousarray(beta.reshape(NT, 128).T).astype(np.float32)
    blockdiag16 = np.kron(np.eye(8, dtype=np.float32), np.ones((GSIZE, 1), np.float32))
    bcast16 = np.ascontiguousarray(blockdiag16.T)
    return dict(wqk_t=wqk_t, bqk=bqk, wv_ext=wv_ext, bv_bc=bv_bc, wp_t=wp_t,
                bp=bp, gamma_t=gamma_t, beta_t=beta_t,
                blockdiag16=blockdiag16, bcast16=bcast16)


def kernel(x, gamma, beta, w_qkv, b_qkv, w_proj, b_proj):
    x = np.asarray(x, dtype=np.float32)
    shared = _prep_shared(np.asarray(w_qkv, np.float32), np.asarray(b_qkv, np.float32),
                          np.asarray(w_proj, np.float32), np.asarray(b_proj, np.float32),
                          np.asarray(gamma, np.float32), np.asarray(beta, np.float32))
    x6 = x.reshape(B, C, N)
    in_maps = [dict(x=np.ascontiguousarray(x6[BPC * i:BPC * (i + 1)]), **shared)
               for i in range(NCORES)]
    if "nc" not in _cached:
        _cached["nc"] = _build()
    res = run_bass_kernel_spmd(_cached["nc"], in_maps, list(range(NCORES)))
    out = np.empty((B, C, N), np.float32)
    for i in range(NCORES):
        out[BPC * i:BPC * (i + 1)] = res.results[i]["out"]
    return out.reshape(B, C, H, W)


# revision 21
# speedup vs baseline: 1.1525x; 1.0244x over previous
"""Trainium2 Bass kernel for an AttentionBlock (GroupNorm + MHA + proj + residual).

Shapes (hardcoded): x (16, 512, 32, 32) f32, 8 heads (ch=64), GN groups=32,
w_qkv (1536, 512), w_proj (512, 512).

Strategy: data-parallel over batch across 8 NeuronCores (2 batches/core, no
collectives). Measured-HW design rules (burst-slope microbenchmarks):
  - float32r matmuls run 1 cyc/row @2.4GHz (218ns for free=512); bf16 and
    fp8 run 1.5 cyc/row, so f32 storage + .bitcast(float32r) at the matmul
    is both fastest and most accurate. Only proj runs bf16 (SBUF pressure).
  - PSUM accumulation chains serialize (~406ns/mm) unless >=2 chains to
    different banks are interleaved instruction-by-instruction (218ns).
  - A stationary (lhsT) tile must serve >=2 consecutive matmuls, then the
    implied ldweights is fully hidden; a fresh lhsT every matmul adds 224ns.
  - ACT exp costs ~250ns + 0.87ns/elem + dep overhead: use [128,1024] tiles.
  - Engine instruction queues are in-order: an instruction whose semaphore
    wait is unsatisfied blocks everything behind it on that engine.

Structure: the attention inner loop (64 slots/batch = 8 heads x 8 k-chunks,
each: 2 score mm -> exp [128,1024] -> 2 attn@v mm, attn@v deferred 3 slots)
is ACT-bound (~1.35us/slot vs ~0.9us PE). All other work — proj+output of
batch b-1 (first, to free the x buffer), x DMA + GN stats/apply of b+1,
then v and qkv of b+1 — is emitted as atomic "filler" units interleaved
into those slots, gated by earliest-slot constraints so a filler's
semaphore wait never blocks the in-order attention stream. qk and xn are
single PERSISTENT tiles (not pool-cycled): each batch overwrites column
ranges and the framework's range-level hazards implement the ring (q/k
o-tile o of batch b+1 may be written once heads 2o,2o+1 of batch b are
done; o=3,7 spill into the first slots of the next heads block).
"""
import numpy as np
import ml_dtypes
from contextlib import ExitStack

import concourse.bass as bass
import concourse.mybir as mybir
import concourse.tile as tile
from concourse import bacc
from concourse.bass_utils import run_bass_kernel_spmd

F32 = mybir.dt.float32
F32R = mybir.dt.float32r
BF16 = mybir.dt.bfloat16
AF = mybir.ActivationFunctionType
OP = mybir.AluOpType

B, C, H, W = 16, 512, 32, 32
N = H * W            # 1024
NHEADS = 8
CH = C // NHEADS     # 64
NGROUPS = 32
GSIZE = C // NGROUPS  # 16 channels per group
EPS = 1e-5
NCORES = 8
BPC = B // NCORES    # batches per core = 2
NT = C // 128        # channel tiles per batch = 4
VW = 520             # per-n-tile v row: 8 heads x (64 ch + 1 ones col)
EXPSCALE = 1.0 / np.sqrt(float(CH))  # folded into the exp activation
SLOTS = NHEADS * 8   # 64 attention slots per batch

_cached = {}


def _build(dbg=False, reps=1, num_devices=NCORES, interleave=True,
           defer=3, p_bufs=6, mode='full', attnonly=False, frontonly=False):
    nc = bacc.Bacc("TRN2", target_bir_lowering=False, debug=False,
                   num_devices=num_devices)

    xd = nc.dram_tensor("x", [BPC, C, N], F32, kind="ExternalInput").ap()
    wqk_d = nc.dram_tensor("wqk_t", [C, 2 * C], F32R, kind="ExternalInput").ap()
    wv_d = nc.dram_tensor("wv_ext", [C, VW], F32R, kind="ExternalInput").ap()
    wp_d = nc.dram_tensor("wp_t", [C, C], BF16, kind="ExternalInput").ap()
    bqk_d = nc.dram_tensor("bqk", [128, 8], F32, kind="ExternalInput").ap()
    bv_d = nc.dram_tensor("bv_bc", [128, VW], F32, kind="ExternalInput").ap()
    bp_d = nc.dram_tensor("bp", [128, NT], F32, kind="ExternalInput").ap()
    gam_d = nc.dram_tensor("gamma_t", [128, NT], F32, kind="ExternalInput").ap()
    bet_d = nc.dram_tensor("beta_t", [128, NT], F32, kind="ExternalInput").ap()
    bd_d = nc.dram_tensor("blockdiag16", [128, 8], F32, kind="ExternalInput").ap()
    bc_d = nc.dram_tensor("bcast16", [8, 128], F32, kind="ExternalInput").ap()
    outd = nc.dram_tensor("out", [BPC, C, N], F32, kind="ExternalOutput").ap()

    with tile.TileContext(nc) as tc, ExitStack() as ctx:
        wpool = ctx.enter_context(tc.tile_pool(name="weights", bufs=1))
        xpool = ctx.enter_context(tc.tile_pool(name="x", bufs=2))
        fixpool = ctx.enter_context(tc.tile_pool(name="fix", bufs=1))
        vpool = ctx.enter_context(tc.tile_pool(name="v", bufs=2))
        hpool = ctx.enter_context(tc.tile_pool(name="h", bufs=2))
        ppool = ctx.enter_context(tc.tile_pool(name="p", bufs=p_bufs))
        opool = ctx.enter_context(tc.tile_pool(name="o", bufs=2))
        small = ctx.enter_context(tc.tile_pool(name="small", bufs=2))
        scr = ctx.enter_context(tc.tile_pool(name="scr", bufs=2))
        ps_sc = ctx.enter_context(tc.tile_pool(name="ps_sc", bufs=2, space="PSUM"))
        ps_h = ctx.enter_context(tc.tile_pool(name="ps_h", bufs=1, space="PSUM"))
        ps_fr = ctx.enter_context(tc.tile_pool(name="ps_fr", bufs=2, space="PSUM"))

        wqk_r, wv_r, wp_r = [], [], []
        for k in range(NT):
            wr = wpool.tile([128, 2 * C], F32R, tag=f"wqk{k}")
            nc.sync.dma_start(wr[:], wqk_d[128 * k:128 * (k + 1), :])
            wqk_r.append(wr)
        for k in range(NT):
            wr = wpool.tile([128, VW], F32R, tag=f"wv{k}")
            nc.sync.dma_start(wr[:], wv_d[128 * k:128 * (k + 1), :])
            wv_r.append(wr)
        for k in range(NT):
            wr = wpool.tile([128, C], BF16, tag=f"wp{k}")
            nc.sync.dma_start(wr[:], wp_d[128 * k:128 * (k + 1), :])
            wp_r.append(wr)

        bqk = wpool.tile([128, 8], F32, tag="bqk")
        nc.sync.dma_start(bqk[:], bqk_d[:])
        bv = wpool.tile([128, VW], F32, tag="bv")
        nc.sync.dma_start(bv[:], bv_d[:])
        bp = wpool.tile([128, NT], F32, tag="bp")
        nc.sync.dma_start(bp[:], bp_d[:])
        gam = wpool.tile([128, NT], F32, tag="gam")
        nc.sync.dma_start(gam[:], gam_d[:])
        bet = wpool.tile([128, NT], F32, tag="bet")
        nc.sync.dma_start(bet[:], bet_d[:])
        bd16 = wpool.tile([128, 8], F32, tag="bd16")
        nc.sync.dma_start(bd16[:], bd_d[:])
        bc16 = wpool.tile([8, 128], F32, tag="bc16")
        nc.sync.dma_start(bc16[:], bc_d[:])

        # persistent single-buffered ring tiles (range-hazard managed)
        qk_t = fixpool.tile([128, 8 * N], F32R, tag="qk", name="qk_ring")
        xn_t = fixpool.tile([128, NT * N], F32R, tag="xn", name="xn_ring")
        pconst = None
        if mode in ("noexp", "constp", "noexp_nonorm"):
            pconst = fixpool.tile([128, N], F32R, tag="pconst", name="pconst")
            nc.gpsimd.dma_start(pconst[:], xd[0, 0:128, :])

        class St:
            pass

        # ---------------- front-end emission pieces ----------------

        def emit_load(st):
            st.x_sb = xpool.tile([128, NT * N], F32, tag="x", name="x_sb")
            for j in range(NT):
                nc.gpsimd.dma_start(st.x_sb[:, N * j:N * (j + 1)],
                                    xd[st.b, 128 * j:128 * (j + 1), :])

        def emit_stats_j(st, j):
            if j == 0:
                st.stat = small.tile([128, 8], F32, tag="stat", name="stat")
            sq = scr.tile([128, N], F32, tag="sq", bufs=1)
            nc.vector.reduce_sum(st.stat[:, j:j + 1],
                                 st.x_sb[:, N * j:N * (j + 1)],
                                 axis=mybir.AxisListType.X)
            nc.vector.scalar_tensor_tensor(
                sq[:], st.x_sb[:, N * j:N * (j + 1)], 1.0,
                st.x_sb[:, N * j:N * (j + 1)],
                op0=OP.bypass, op1=OP.mult,
                accum_out=st.stat[:, 4 + j:5 + j])

        def emit_gn_small(st):
            # group stats + Newton rsqrt + per-channel A/B (all tiny DVE ops)
            ps_st = ps_fr.tile([8, 8], F32, tag="fr")
            nc.tensor.matmul(ps_st[:], bd16[:], st.stat[:], start=True, stop=True)
            inv = 1.0 / (GSIZE * N)
            mean8 = small.tile([8, 8], F32, tag="mean8")
            nc.vector.tensor_scalar_mul(mean8[:, 0:4], ps_st[:, 0:4], inv)
            ex2 = small.tile([8, 4], F32, tag="ex2")
            nc.vector.tensor_scalar_mul(ex2[:], ps_st[:, 4:8], inv)
            m2 = small.tile([8, 4], F32, tag="m2")
            nc.vector.tensor_mul(m2[:], mean8[:, 0:4], mean8[:, 0:4])
            veps = small.tile([8, 4], F32, tag="veps")
            nc.vector.scalar_tensor_tensor(veps[:], ex2[:], EPS, m2[:],
                                           op0=OP.add, op1=OP.subtract)
            r_cur = small.tile([8, 4], F32, tag="r0")
            nc.vector.tensor_scalar(r_cur[:], veps[:], -0.5, 1.5,
                                    op0=OP.mult, op1=OP.add)
            for it in range(3):
                t1 = small.tile([8, 4], F32, tag=f"nt1_{it}")
                nc.vector.tensor_mul(t1[:], r_cur[:], r_cur[:])
                t2 = small.tile([8, 4], F32, tag=f"nt2_{it}")
                nc.vector.scalar_tensor_tensor(t2[:], t1[:], -0.5, veps[:],
                                               op0=OP.mult, op1=OP.mult)
                t3 = small.tile([8, 4], F32, tag=f"nt3_{it}")
                nc.vector.tensor_scalar_add(t3[:], t2[:], 1.5)
                r_nxt = small.tile([8, 4], F32, tag=f"nr_{it}")
                nc.vector.tensor_mul(r_nxt[:], r_cur[:], t3[:])
                r_cur = r_nxt
            nc.vector.tensor_copy(mean8[:, 4:8], r_cur[:])
            ps_bc = ps_fr.tile([128, 8], F32, tag="fr")
            nc.tensor.matmul(ps_bc[:], bc16[:], mean8[:], start=True, stop=True)
            st.A_ch = small.tile([128, NT], F32, tag="A_ch", name="A_ch")
            nc.vector.tensor_mul(st.A_ch[:], gam[:], ps_bc[:, 4:8])
            tB = small.tile([128, NT], F32, tag="tB")
            nc.vector.tensor_mul(tB[:], ps_bc[:, 0:4], st.A_ch[:])
            st.B_ch = small.tile([128, NT], F32, tag="B_ch", name="B_ch")
            nc.vector.scalar_tensor_tensor(st.B_ch[:], tB[:], -1.0, bet[:],
                                           op0=OP.mult, op1=OP.add)

        def emit_apply_j(st, j):
            nc.vector.tensor_scalar(xn_t[:, N * j:N * (j + 1)],
                                    st.x_sb[:, N * j:N * (j + 1)],
                                    st.A_ch[:, j:j + 1], st.B_ch[:, j:j + 1],
                                    op0=OP.mult, op1=OP.add)

        def emit_v_nt(st, ntile):
            # v^T n-tile: out [128 n, 520 c] (8 heads x 64 ch + zero-weight
            # ones cols whose 1.0 comes from the bias), 4 accumulated k-tiles
            # as two interleaved chains (free 512 + free 8)
            if ntile == 0:
                st.vv = vpool.tile([128, 8 * VW], F32R, tag="vv", name="vv")
            pva = ps_fr.tile([128, 512], F32, tag="fr", name=f"pva{ntile}")
            pvb = ps_fr.tile([128, 8], F32, tag="fr", name=f"pvb{ntile}")
            for k in range(NT):
                xsl = xn_t[:, N * k + 128 * ntile:N * k + 128 * (ntile + 1)]
                nc.tensor.matmul(pva[:], xsl, wv_r[k][:, 0:512],
                                 start=(k == 0), stop=(k == NT - 1))
                nc.tensor.matmul(pvb[:], xsl, wv_r[k][:, 512:VW],
                                 start=(k == 0), stop=(k == NT - 1))
            nc.vector.scalar_tensor_tensor(
                st.vv[:, VW * ntile:VW * ntile + 512], pva[:], 1.0,
                bv[:, 0:512], op0=OP.bypass, op1=OP.add)
            nc.vector.scalar_tensor_tensor(
                st.vv[:, VW * ntile + 512:VW * (ntile + 1)], pvb[:], 1.0,
                bv[:, 512:VW], op0=OP.bypass, op1=OP.add)

        def emit_qkv_o(st, o):
            # q/k o-tile: out [128 oc, 1024 n] via 2 interleaved nh chains
            pq = [ps_fr.tile([128, 512], F32, tag="fr", name=f"pq{o}_{nh}")
                  for nh in range(2)]
            for k in range(NT):
                for nh in range(2):
                    nc.tensor.matmul(
                        pq[nh][:],
                        (wqk_r[k][:, 128 * o:128 * (o + 1)]),
                        (xn_t[:, N * k + 512 * nh:N * k + 512 * (nh + 1)]),
                        start=(k == 0), stop=(k == NT - 1))
            for nh in range(2):
                nc.vector.tensor_scalar_add(
                    qk_t[:, N * o + 512 * nh:N * o + 512 * (nh + 1)],
                    pq[nh][:], bqk[:, o:o + 1])

        def emit_proj_o(st, o):
            pp = [ps_fr.tile([128, 512], F32, tag="fr", name=f"pp{o}_{nh}")
                  for nh in range(2)]
            for k in range(NT):
                for nh in range(2):
                    nc.tensor.matmul(
                        pp[nh][:],
                        wp_r[k][:, 128 * o:128 * (o + 1)],
                        st.hall[:, N * k + 512 * nh:N * k + 512 * (nh + 1)],
                        start=(k == 0), stop=(k == NT - 1))
            for nh in range(2):
                ot = opool.tile([128, 512], F32, tag="ot")
                nc.vector.scalar_tensor_tensor(
                    ot[:], pp[nh][:], bp[:, o:o + 1],
                    st.x_sb[:, N * o + 512 * nh:N * o + 512 * (nh + 1)],
                    op0=OP.add, op1=OP.add)
                nc.sync.dma_start(
                    outd[st.b, 128 * o:128 * (o + 1), 512 * nh:512 * (nh + 1)],
                    ot[:])

        def emit_front_straight(st):
            emit_load(st)
            for j in range(NT):
                emit_stats_j(st, j)
            emit_gn_small(st)
            for j in range(NT):
                emit_apply_j(st, j)
            for ntile in range(8):
                emit_v_nt(st, ntile)
            for o in range(8):
                emit_qkv_o(st, o)

        # ---------------- filler units ----------------
        # unit = (min_slot_within_this_heads_block, weight, closure)

        def proj_units(st):
            return [(1 + 2 * o, 8.0, lambda o=o: emit_proj_o(st, o))
                    for o in range(NT)]

        def front_units(st):
            """Units for batch st's front-end, run under heads(prev).
            Returns (units, late_units); late_units go into the NEXT heads
            block (qk o-tiles 3 and 7 are still being read until the last
            head of heads(prev))."""
            u = [(9, 0.0, lambda: emit_load(st))]
            for j in range(NT):
                u.append((14 + j, 0.7, lambda j=j: emit_stats_j(st, j)))
            u.append((19, 1.0, lambda: emit_gn_small(st)))
            for j in range(NT):
                u.append((21 + j, 0.7, lambda j=j: emit_apply_j(st, j)))
            for ntile in range(8):
                u.append((26 + 2 * ntile, 4.0,
                          lambda n_=ntile: emit_v_nt(st, n_)))
            # qk ring gates: o-tile o writable after heads 2o,2o+1 done
            for o in range(3):
                for oo in (o, o + 4):
                    u.append((max((2 * o + 2) * 8, 27 + 2 * o), 8.0,
                              lambda oo=oo: emit_qkv_o(st, oo)))
            late = [(1 + 2 * i, 8.0, lambda oo=oo: emit_qkv_o(st, oo))
                    for i, oo in enumerate((3, 7))]
            return u, late

        # ---------------- attention (the slot loop) ----------------

        def emit_heads(st, fillers):
            DEFER = defer
            skipnorm = mode in ("nonorm", "noexp_nonorm")
            fq = sorted(fillers, key=lambda t: t[0])
            fi = 0
            st.hall = hpool.tile([128, NT * N], BF16, tag="hall", name="hall")
            pend = []  # (head, j, p_tile) awaiting attn@v
            phs = {}

            def emit_av(h, j, p_t):
                if j == 0:
                    phs[h] = ps_h.tile([65, N], F32, tag="hacc",
                                       name=f"phs{h}")
                for th in range(2):
                    nc.tensor.matmul(
                        phs[h][:, 512 * th:512 * (th + 1)],
                        (st.vv[:, VW * j + 65 * h:VW * j + 65 * h + 65]),
                        (p_t[:, 512 * th:512 * (th + 1)]),
                        start=(j == 0), stop=(j == 7))

            def emit_norm(h):
                ph = phs.pop(h)
                if skipnorm:
                    nc.vector.tensor_copy(
                        st.hall[64 * (h % 2):64 * (h % 2) + 64,
                                N * (h // 2):N * (h // 2 + 1)],
                        ph[0:64, :])
                    return
                rec = small.tile([1, N], F32, tag="rec")
                nc.vector.reciprocal(rec[:], ph[64:65, :])
                rb = scr.tile([64, N], F32, tag="rb", bufs=1)
                nc.gpsimd.partition_broadcast(rb[:], rec[:])
                nc.vector.tensor_mul(
                    st.hall[64 * (h % 2):64 * (h % 2) + 64,
                            N * (h // 2):N * (h // 2 + 1)],
                    ph[0:64, :], rb[:])

            for h in range(NHEADS):
                pb = 64 * (h % 2)
                q_base = N * (h // 2)
                k_base = 4 * N + N * (h // 2)
                for j2 in range(0, 8, 2):
                    # run of 4 score matmuls (two slots' worth): fewer PE
                    # array tile-config switches than alternating sc/av
                    for j in (j2, j2 + 1):
                        sc = ps_sc.tile([128, N], F32, tag="sc")
                        for th in range(2):
                            nc.tensor.matmul(
                                sc[:, 512 * th:512 * (th + 1)],
                                (qk_t[pb:pb + 64, k_base + 128 * j:
                                        k_base + 128 * (j + 1)]),
                                (qk_t[pb:pb + 64, q_base + 512 * th:
                                        q_base + 512 * (th + 1)]),
                                start=True, stop=True,
                                tile_position=(pb, 0))
                        if mode not in ("noexp", "noexp_nonorm"):
                            p_t = ppool.tile([128, N], F32R, tag="p")
                            nc.scalar.activation(p_t[:], sc[:], AF.Exp,
                                                 scale=EXPSCALE)
                        if mode in ("noexp", "constp", "noexp_nonorm"):
                            pend.append((h, j, pconst))
                        else:
                            pend.append((h, j, p_t))
                    slot = 8 * h + j2 + 1
                    while pend and (8 * pend[0][0] + pend[0][1] + DEFER
                                    <= slot):
                        ah, aj, ap_t = pend.pop(0)
                        emit_av(ah, aj, ap_t)
                        if aj == 7:
                            emit_norm(ah)
                    if j2 == 6:
                        # force-drain this head's remaining attn@v now so the
                        # normalize chain gets a multi-slot window before the
                        # single phs buffer is needed by the next head
                        while pend and pend[0][0] == h:
                            ah, aj, ap_t = pend.pop(0)
                            emit_av(ah, aj, ap_t)
                            if aj == 7:
                                emit_norm(ah)
                    budget = 5.0
                    while fi < len(fq) and budget > 0:
                        ms, wgt, fn = fq[fi]
                        if ms > slot:
                            break
                        fn()
                        budget -= wgt
                        fi += 1
            while pend:
                ah, aj, ap_t = pend.pop(0)
                emit_av(ah, aj, ap_t)
                if aj == 7:
                    emit_norm(ah)
            while fi < len(fq):
                fq[fi][2]()
                fi += 1

        # ---------------- batch pipeline ----------------
        bs = [b for _ in range(reps) for b in range(BPC)]
        sts = []
        st0 = St()
        st0.b = bs[0]
        emit_front_straight(st0)  # prologue
        sts.append(st0)
        carry = []  # late units from front(b+1) for the next heads block
        if attnonly:
            for idx in range(len(bs)):
                emit_heads(st0, [])
            nc.compile()
            return nc
        if frontonly:
            for idx in range(1, len(bs)):
                sti = St()
                sti.b = bs[idx]
                emit_front_straight(sti)
            nc.compile()
            return nc
        for idx in range(len(bs)):
            st = sts[idx]
            fillers = list(carry)
            carry = []
            if idx >= 1:
                fillers += proj_units(sts[idx - 1])
            if idx + 1 < len(bs):
                nxt = St()
                nxt.b = bs[idx + 1]
                sts.append(nxt)
                u, late = front_units(nxt)
                fillers += u
                carry = late
            if interleave:
                emit_heads(st, fillers)
            else:
                emit_heads(st, [])
                for _ms, _w, fn in sorted(fillers + carry,
                                          key=lambda t: t[0]):
                    fn()
                carry = []
        for fn in [t[2] for t in carry]:
            fn()
        last = sts[-1]
        for o in range(NT):
            emit_proj_o(last, o)

    nc.compile()
    return nc


def _prep_shared(w_qkv, b_qkv, w_proj, b_proj, gamma, beta):
    r = np.arange(3 * C).reshape(NHEADS, 3, CH)
    idx_q, idx_k, idx_v = r[:, 0].ravel(), r[:, 1].ravel(), r[:, 2].ravel()
    wqk_t = np.ascontiguousarray(
        np.concatenate([w_qkv[idx_q], w_qkv[idx_k]], axis=0).T).astype(np.float32)
    bqk_full = np.concatenate([b_qkv[idx_q], b_qkv[idx_k]])
    bqk = np.ascontiguousarray(bqk_full.reshape(8, 128).T).astype(np.float32)

    wv = w_qkv[idx_v]
    bv_src = b_qkv[idx_v]
    wv_ext = np.zeros((C, VW), np.float32)
    bv_ext = np.zeros((VW,), np.float32)
    for h in range(NHEADS):
        wv_ext[:, 65 * h:65 * h + CH] = wv[CH * h:CH * (h + 1), :].T
        bv_ext[65 * h:65 * h + CH] = bv_src[CH * h:CH * (h + 1)]
        bv_ext[65 * h + CH] = 1.0
    bv_bc = np.ascontiguousarray(np.broadcast_to(bv_ext, (128, VW))).astype(np.float32)

    wp_t = np.ascontiguousarray(w_proj.T).astype(ml_dtypes.bfloat16)
    bp = np.ascontiguousarray(b_proj.reshape(NT, 128).T).astype(np.float32)
    gamma_t = np.ascontiguousarray(gamma.reshape(NT, 128).T).astype(np.float32)
    beta_t = np.ascontiguousarray(beta.reshape(NT, 128).T).astype(np.float32)
    blockdiag16 = np.kron(np.eye(8, dtype=np.float32), np.ones((GSIZE, 1), np.float32))
    bcast16 = np.ascontiguousarray(blockdiag16.T)
    return dict(wqk_t=wqk_t, bqk=bqk, wv_ext=wv_ext, bv_bc=bv_bc, wp_t=wp_t,
                bp=bp, gamma_t=gamma_t, beta_t=beta_t,
                blockdiag16=blockdiag16, bcast16=bcast16)


def kernel(x, gamma, beta, w_qkv, b_qkv, w_proj, b_proj):
    x = np.asarray(x, dtype=np.float32)
    shared = _prep_shared(np.asarray(w_qkv, np.float32), np.asarray(b_qkv, np.float32),
                          np.asarray(w_proj, np.float32), np.asarray(b_proj, np.float32),
                          np.asarray(gamma, np.float32), np.asarray(beta, np.float32))
    x6 = x.reshape(B, C, N)
    in_maps = [dict(x=np.ascontiguousarray(x6[BPC * i:BPC * (i + 1)]), **shared)
               for i in range(NCORES)]
    if "nc" not in _cached:
        _cached["nc"] = _build()
    res = run_bass_kernel_spmd(_cached["nc"], in_maps, list(range(NCORES)))
    out = np.empty((B, C, N), np.float32)
    for i in range(NCORES):
        out[BPC * i:BPC * (i + 1)] = res.results[i]["out"]
    return out.reshape(B, C, H, W)
